# revision 3
# baseline (speedup 1.0000x reference)
"""MoE layer (top-2 of 8 experts) on 8 Trainium2 NeuronCores — v2.

Strategy (expert x F-slice sharding for perfect load balance):
  * Host computes gating (softmax + top-2) exactly as the reference, sorts
    the N*K = 16384 (token, expert) pairs by expert id, and ships the
    dispatched token matrix xT [D, 16384] (bf16, transposed) to EVERY core.
  * Core c holds W1[:, :, c*512:(c+1)*512] and W2[:, c*512:(c+1)*512, :]
    for ALL 8 experts (16.8 MB bf16, SBUF-resident) and computes the
    partial FFN contribution of its 512-wide f-slice for ALL 16384 slots:
        part_c[s] = relu(x[s] @ W1[e_s][:, fs]) @ W2[e_s][fs, :]
    Work per core is exactly 16384/8-token-equivalents regardless of how
    tokens route --> no capacity padding, perfect balance.
  * Host sums the 8 partials and combines with the top-2 gates.

Device kernel (per core, identical program on all 8 cores):
  for each chunk (expert e, tokens [s0, s0+L), L <= 512):
    mm1: ph[fc][f128, L]  += w1[e][ki, fc].T @ xT[ki, s0:s0+L]   (ki = 0..7)
    relu: h[fc] = relu(ph[fc])  (bf16, SBUF)       4 PSUM banks
    mm2: po[nd][tok128, 512] += h[kf, tm].T @ w2[e][kf, nd]      (kf = 0..3)
    evac po -> o_sb (bf16) -> DMA out rows [s0+tm*128 ...)       4 PSUM banks
"""

import time

import numpy as np
import ml_dtypes

import concourse.bass as bass
import concourse.mybir as mybir
import concourse.tile as tile
from concourse import bacc
from concourse.bass_utils import run_bass_kernel_spmd

N, D, F, E, TOPK = 8192, 1024, 4096, 8, 2
P = 128
NCORES = 8
FS = F // NCORES          # 512 f-columns per core
KD = D // P               # 8 k-tiles over d_model
KF = FS // P              # 4 k-tiles over the f-slice
S = N * TOPK              # 16384 dispatched slots
CHUNK = 512               # tokens per chunk (PSUM-bank limited)

BF16 = mybir.dt.bfloat16
F32 = mybir.dt.float32

_program_cache: dict[tuple, "bass.Bass"] = {}
LAST_RESULTS = None
TRACE = False


def _chunk_list(seg_lens):
    """[(expert, start, len)] covering the expert-sorted slot space exactly."""
    chunks = []
    s = 0
    for e, L in enumerate(seg_lens):
        t = 0
        while t < L:
            c = min(CHUNK, L - t)
            chunks.append((e, s + t, c))
            t += c
        s += L
    return chunks


def _build_program(seg_lens: tuple, bench_iters: int = 1,
                   relu_dve: bool = False) -> "bass.Bass":
    chunks = _chunk_list(seg_lens)
    assert sum(seg_lens) == S

    nc = bacc.Bacc("TRN2", target_bir_lowering=False, debug=False,
                   num_devices=NCORES)
    xT = nc.dram_tensor("xT", [D, S], BF16, kind="ExternalInput")
    w1 = nc.dram_tensor("w1", [E, D, FS], BF16, kind="ExternalInput")
    w2 = nc.dram_tensor("w2", [E, FS, D], BF16, kind="ExternalInput")
    out = nc.dram_tensor("out", [S, D], BF16, kind="ExternalOutput")

    # partition-major views: one batched DMA per chunk / per expert weight
    xT_p = xT[:].rearrange("(ki p) s -> p ki s", p=P)
    w1_p = w1[:].rearrange("e (ki p) f -> e p ki f", p=P)
    w2_p = w2[:].rearrange("e (kf p) d -> e p kf d", p=P)

    with tile.TileContext(nc) as tc:
        with (
            tc.tile_pool(name="wpool", bufs=1) as wpool,
            tc.tile_pool(name="xpool", bufs=3) as xpool,
            tc.tile_pool(name="hpool", bufs=2) as hpool,
            tc.tile_pool(name="opool", bufs=2) as opool,
            tc.tile_pool(name="ph_pool", bufs=1, space="PSUM") as ph_pool,
            tc.tile_pool(name="po_pool", bufs=2, space="PSUM") as po_pool,
        ):
            # resident weights, one tile + one DMA per (expert, which) so the
            # first chunks of expert e only wait on expert e's two transfers
            w1_sb = [wpool.tile([P, KD, FS], BF16, name=f"w1_{e}")
                     for e in range(E)]
            w2_sb = [wpool.tile([P, KF, D], BF16, name=f"w2_{e}")
                     for e in range(E)]
            loaded = set()

            def load_weights(e, which=(1, 2)):
                if 1 in which and (e, 1) not in loaded:
                    loaded.add((e, 1))
                    nc.sync.dma_start(w1_sb[e], w1_p[e])
                if 2 in which and (e, 2) not in loaded:
                    loaded.add((e, 2))
                    nc.sync.dma_start(w2_sb[e], w2_p[e])

            def do_chunk(e, s0, L, after_x=None):
                TM = (L + P - 1) // P
                xc = xpool.tile([P, KD, CHUNK], BF16, name="xc", tag="xc")
                nc.sync.dma_start(xc[:, :, :L], xT_p[:, :, s0:s0 + L])
                if after_x is not None:
                    after_x()   # emit follow-on weight DMAs behind the x chunk

                h = hpool.tile([P, KF, CHUNK], BF16, name="h", tag="h")
                for fc in range(KF):
                    ph = ph_pool.tile([P, CHUNK], F32, name="ph",
                                      tag=f"ph{fc}")
                    for ki in range(KD):
                        nc.tensor.matmul(
                            ph[:, :L],
                            lhsT=w1_sb[e][:, ki, fc * P:(fc + 1) * P],
                            rhs=xc[:, ki, :L],
                            start=(ki == 0),
                            stop=(ki == KD - 1),
                        )
                    if relu_dve and fc == KF - 1:
                        # last f-tile: DVE finishes sooner than the queued-up
                        # ACT, so mm2's kf=3 matmuls never wait on the relu
                        nc.vector.tensor_scalar_max(h[:, fc, :L], ph[:, :L],
                                                    0.0)
                    else:
                        nc.scalar.activation(h[:, fc, :L], ph[:, :L],
                                             mybir.ActivationFunctionType.Relu)

                o_sb = opool.tile([P, 4, D], BF16, name="o_sb", tag="o_sb")
                for tm in range(TM):
                    m = min(P, L - tm * P)
                    po = [po_pool.tile([P, 512], F32, name=f"po{nd}",
                                       tag=f"po{nd}") for nd in range(2)]
                    for kf in range(KF):
                        for nd in range(2):
                            nc.tensor.matmul(
                                po[nd][:m, :],
                                lhsT=h[:, kf, tm * P:tm * P + m],
                                rhs=w2_sb[e][:, kf, nd * 512:(nd + 1) * 512],
                                start=(kf == 0),
                                stop=(kf == KF - 1),
                            )
                    for nd in range(2):
                        nc.vector.tensor_copy(
                            o_sb[:m, tm, nd * 512:(nd + 1) * 512], po[nd][:m, :]
                        )
                if L == 4 * P:
                    nc.sync.dma_start(
                        out[s0:s0 + L, :].rearrange("(tm p) d -> p tm d", p=P),
                        o_sb,
                    )
                else:
                    for tm in range(TM):
                        m = min(P, L - tm * P)
                        nc.sync.dma_start(out[s0 + tm * P:s0 + tm * P + m, :],
                                          o_sb[:m, tm, :])

            if bench_iters > 1:
                for e in range(E):
                    load_weights(e)
                with tc.For_i(0, bench_iters, 1):
                    for (e, s0, L) in chunks:
                        do_chunk(e, s0, L)
            else:
                # stream weights one expert ahead of the compute so the
                # first matmul only waits on w1[0] + one x chunk (~2 MB)
                load_weights(0, which=(1,))

                def first_chunk_weights():
                    load_weights(0, which=(2,))
                    load_weights(1)

                for ci, (e, s0, L) in enumerate(chunks):
                    if ci > 0 and e != chunks[ci - 1][0] and e + 1 < E:
                        load_weights(e + 1)
                    do_chunk(e, s0, L,
                             after_x=first_chunk_weights if ci == 0 else None)
    nc.compile()
    return nc


def _gate_and_dispatch(x, w_gate):
    """Replicates the reference gating exactly (fp32): softmax + top-2."""
    logits = x.astype(np.float32) @ w_gate.astype(np.float32)        # [N, E]
    m = logits.max(-1, keepdims=True)
    p = np.exp(logits - m)
    probs = p / p.sum(-1, keepdims=True)
    # jax.lax.top_k: descending, ties broken by lower index -> stable argsort
    tk_idx = np.argsort(-probs, axis=1, kind="stable")[:, :TOPK]
    tk_vals = np.take_along_axis(probs, tk_idx, axis=1)
    tk_gates = tk_vals / (tk_vals.sum(-1, keepdims=True) + 1e-9)
    return tk_idx, tk_gates


def kernel(x, w_gate, W1, W2):
    global LAST_RESULTS
    x = np.asarray(x, dtype=np.float32)
    w_gate = np.asarray(w_gate, dtype=np.float32)
    W1 = np.asarray(W1, dtype=np.float32)
    W2 = np.asarray(W2, dtype=np.float32)
    n_tok = x.shape[0]

    tk_idx, tk_gates = _gate_and_dispatch(x, w_gate)

    eid = tk_idx.reshape(-1).astype(np.int64)
    seg_lens = tuple(int(v) for v in np.bincount(eid, minlength=E))
    order = np.argsort(eid, kind="stable")
    slot_of = np.empty(n_tok * TOPK, np.int64)
    slot_of[order] = np.arange(n_tok * TOPK)
    tok_of_flat = np.repeat(np.arange(n_tok), TOPK)

    # dispatched tokens, transposed: xT[:, slot] = x[token(slot)]
    xb = x.astype(ml_dtypes.bfloat16)
    xT_disp = np.ascontiguousarray(xb[tok_of_flat[order]].T)   # [D, S]

    in_maps = []
    for c in range(NCORES):
        fs = slice(c * FS, (c + 1) * FS)
        in_maps.append({
            "xT": xT_disp,
            "w1": np.ascontiguousarray(W1[:, :, fs]).astype(ml_dtypes.bfloat16),
            "w2": np.ascontiguousarray(W2[:, fs, :]).astype(ml_dtypes.bfloat16),
        })

    nc = _program_cache.get(seg_lens)
    if nc is None:
        nc = _build_program(seg_lens, relu_dve=True)
        _program_cache[seg_lens] = nc

    try:
        res = run_bass_kernel_spmd(nc, in_maps, core_ids=list(range(NCORES)),
                                   trace=TRACE)
    except Exception:
        time.sleep(20)
        res = run_bass_kernel_spmd(nc, in_maps, core_ids=list(range(NCORES)),
                                   trace=TRACE)
    LAST_RESULTS = res

    # combine: sum f-slice partials, then y[n] = sum_k gates[n,k] * O[slot]
    O = np.zeros((n_tok * TOPK, D), np.float32)
    for c in range(NCORES):
        O += np.asarray(res.results[c]["out"]).astype(np.float32)
    flat_rows = O[slot_of]                              # [n_tok*K, D]
    y = (tk_gates.reshape(-1, 1) * flat_rows).reshape(n_tok, TOPK, D).sum(axis=1)
    return y.astype(np.float32)
